# revision 10
# baseline (speedup 1.0000x reference)
"""Causal single-head attention on 8 NeuronCores (Trainium2, Bass/Tile).

Problem: x[16,4096,128] fp32; Wq/Wk/Wv[128,128]; y = softmax(mask(QK^T/sqrt(128))) @ V.
Sharding: data-parallel over batch, 2 batches per core, no collectives.

Per-batch on-core plan (S^T orientation so the PV matmul needs no P transpose):
  xT = PE-transpose(x)            [d, L]   fp32 -> rounded to fp32r
  qT = Wq^T @ xT, kT = Wk^T @ xT  [h, L]   fp32r
  V  = xT^T @ Wv                  [l, h]   fp32r
  per 512-wide q-block J, per 128-wide k-tile i <= diag:
    S^T = kT_i^T @ qT_J           [128k, Nq]  (PSUM fp32)
    diag tiles: S^T[:, :128] += causal_mask (-1e30 strict lower)
    P^T = exp(scale * S^T)        ACT, PSUM->SBUF, fp32r
    sums += ones^T @ P^T          [1, Nq] PSUM accum
    O^T  += V_i^T @ P^T           [h, Nq] PSUM accum
  transpose sums -> [128,1] x4, reciprocal, transpose O^T -> O, scale rows, DMA out.
"""
import sys

if '/opt/trn_rl_repo' not in sys.path:
    sys.path.insert(0, '/opt/trn_rl_repo')

import numpy as np

B, L, D, H = 16, 4096, 128, 128
NCORES = 8
BPC = B // NCORES          # batches per core
QB = 512                   # q block width
NQB = L // QB              # 8 q blocks
KT = 128                   # k tile width
NKT = L // KT              # 32 k tiles
CHUNK = 512                # phase-A l-chunk
NCHUNK = L // CHUNK        # 8
SCALE = float(1.0 / np.sqrt(H))
NEG = -1.0e30

_cache = {}


def _build(reps=0):
    import contextlib

    import concourse.mybir as mybir
    import concourse.tile as tile
    from concourse import bacc

    f32 = mybir.dt.float32
    f32r = mybir.dt.float32r
    f16 = mybir.dt.float16
    EXP = mybir.ActivationFunctionType.Exp
    CPY = mybir.ActivationFunctionType.Copy

    nc = bacc.Bacc("TRN2", target_bir_lowering=False, debug=False,
                   num_devices=NCORES)
    x_ap = nc.dram_tensor("x", [BPC, L, D], f32, kind="ExternalInput").ap()
    wq_ap = nc.dram_tensor("Wq", [D, H], f32, kind="ExternalInput").ap()
    wk_ap = nc.dram_tensor("Wk", [D, H], f32, kind="ExternalInput").ap()
    wv_ap = nc.dram_tensor("Wv", [D, H], f32, kind="ExternalInput").ap()
    id_ap = nc.dram_tensor("ident", [128, 128], f32, kind="ExternalInput").ap()
    mk_ap = nc.dram_tensor("mask", [128, 128], f32, kind="ExternalInput").ap()
    y_ap = nc.dram_tensor("y", [BPC, L, H], f32, kind="ExternalOutput").ap()

    with tile.TileContext(nc) as tc:
        with (
            tc.tile_pool(name="const", bufs=1) as constp,
            tc.tile_pool(name="xchunk", bufs=3) as xchp,
            tc.tile_pool(name="xt", bufs=3) as xtp,
            tc.tile_pool(name="qkv", bufs=BPC) as qkvp,
            tc.tile_pool(name="pt", bufs=3) as ptp,
            tc.tile_pool(name="otsb", bufs=2) as otsbp,
            tc.tile_pool(name="smsb", bufs=2) as smsbp,
            tc.tile_pool(name="ysb", bufs=4) as yp,
            tc.tile_pool(name="ps_mm", bufs=2, space="PSUM") as ps_mm,
            tc.tile_pool(name="ps_ot", bufs=2, space="PSUM") as ps_ot,
            tc.tile_pool(name="ps_small", bufs=2, space="PSUM") as ps_small,
            tc.tile_pool(name="ps_sums", bufs=2, space="PSUM") as ps_sums,
        ):
            # ---- constants ----
            ident = constp.tile([128, 128], f32, tag="ident")
            nc.sync.dma_start(ident[:], id_ap[:])
            mask = constp.tile([128, 128], f32, tag="mask")
            nc.sync.dma_start(mask[:], mk_ap[:])
            w_f = {}
            w_r = {}
            for name, ap in (("q", wq_ap), ("k", wk_ap), ("v", wv_ap)):
                wf = constp.tile([128, 128], f32, tag=f"w{name}f")
                nc.sync.dma_start(wf[:], ap[:])
                wr = constp.tile([128, 128], f32r, tag=f"w{name}r")
                nc.vector.tensor_copy(wr[:], wf[:])
                w_f[name] = wf
                w_r[name] = wr
            ones_f = constp.tile([128, 1], f32, tag="ones_f")
            nc.gpsimd.memset(ones_f[:], 1.0)
            ones_r = constp.tile([128, 1], f32r, tag="ones_r")
            nc.vector.tensor_copy(ones_r[:], ones_f[:])
            ones_h = constp.tile([1, 1], f16, tag="ones_h")
            nc.gpsimd.memset(ones_h[:], 1.0)

            rep_ctx = tc.For_i(0, reps, 1) if reps else contextlib.nullcontext()
            rep_stack = contextlib.ExitStack()
            rep_stack.enter_context(rep_ctx)

            # ---- per-batch tensors ----
            qT = {}
            kT = {}
            Vn = {}
            for b in range(BPC):
                qT[b] = qkvp.tile([128, L], f32r, tag="qT", name=f"qT{b}")
                kT[b] = qkvp.tile([128, L], f32r, tag="kT", name=f"kT{b}")
                Vn[b] = qkvp.tile([128, L], f32r, tag="V", name=f"V{b}")

            # ---- phase A: transpose + projections ----
            for b in range(BPC):
                xv = x_ap[b].rearrange("(n p) d -> p n d", p=128)
                for c in range(NCHUNK):
                    xch = xchp.tile([128, 4, 128], f32, tag="xch")
                    nc.sync.dma_start(xch[:], xv[:, 4 * c:4 * c + 4, :])
                    xt = xtp.tile([128, CHUNK], f32r, tag="xt")
                    for n in range(4):
                        tp = ps_small.tile([128, 128], f32, tag="small")
                        nc.tensor.transpose(tp[:], xch[:, n, :], ident[:])
                        nc.vector.tensor_copy(xt[:, 128 * n:128 * (n + 1)], tp[:])
                    # q^T, k^T chunks: [h, CHUNK]; copies on ACT (idle here)
                    for name, dst in (("q", qT[b]), ("k", kT[b])):
                        pp = ps_mm.tile([128, CHUNK], f32, tag="mm")
                        nc.tensor.matmul(pp[:], w_r[name][:], xt[:],
                                         start=True, stop=True)
                        nc.scalar.activation(
                            dst[:, CHUNK * c:CHUNK * (c + 1)], pp[:], CPY)
                    # V tiles: [l,h] per 128-l sub-tile
                    for n in range(4):
                        vp = ps_small.tile([128, 128], f32, tag="small")
                        nc.tensor.matmul(vp[:], xt[:, 128 * n:128 * (n + 1)],
                                         w_r["v"][:], start=True, stop=True)
                        nc.vector.tensor_copy(
                            Vn[b][:, CHUNK * c + 128 * n:CHUNK * c + 128 * (n + 1)],
                            vp[:])

            # ---- phase B: attention ----
            for b in range(BPC):
                yv = y_ap[b].rearrange("(n p) h -> p n h", p=128)
                for J in range(NQB):
                    nkt = 4 * J + 4
                    ot = ps_ot.tile([128, QB], f32, tag="ot")
                    sm = ps_sums.tile([1, QB], f32, tag="sums")
                    for i in range(nkt):
                        qoff = max(0, 128 * (i - 4 * J))
                        N = QB - qoff
                        st = ps_mm.tile([128, QB], f32, tag="mm")
                        nc.tensor.matmul(
                            st[:, :N],
                            kT[b][:, KT * i:KT * (i + 1)],
                            qT[b][:, QB * J + qoff:QB * (J + 1)],
                            start=True, stop=True)
                        if i >= 4 * J:
                            nc.vector.tensor_add(st[:, :128], st[:, :128],
                                                 mask[:])
                        pt = ptp.tile([128, QB], f32r, tag="pt")
                        nc.scalar.activation(pt[:, :N], st[:, :N], EXP,
                                             scale=SCALE)
                        first = (i == 0)
                        last = (i == nkt - 1)
                        nc.tensor.matmul(sm[0:1, qoff:], ones_r[:, 0:1],
                                         pt[:, :N], start=first, stop=last,
                                         skip_group_check=True)
                        nc.tensor.matmul(ot[:, qoff:],
                                         Vn[b][:, KT * i:KT * (i + 1)],
                                         pt[:, :N], start=first, stop=last,
                                         skip_group_check=True)
                    # sums -> per-partition reciprocals (via fp16 transpose mm)
                    smr = smsbp.tile([1, QB], f16, tag="smsb")
                    nc.vector.tensor_copy(smr[:], sm[:])
                    stp = ps_small.tile([128, 4], f32, tag="small",
                                        padded_shape=[128, 128])
                    for j in range(4):
                        nc.tensor.matmul(stp[:, j:j + 1],
                                         smr[0:1, 128 * j:128 * (j + 1)],
                                         ones_h[0:1, 0:1],
                                         start=True, stop=True,
                                         skip_group_check=True)
                    rcp = smsbp.tile([128, 4], f32, tag="rcp")
                    nc.vector.reciprocal(rcp[:], stp[:])
                    # O^T -> O, normalize, store
                    otsb = otsbp.tile([128, QB], f32, tag="otsb")
                    nc.vector.tensor_copy(otsb[:], ot[:])
                    for j in range(4):
                        op = ps_small.tile([128, 128], f32, tag="small")
                        nc.tensor.transpose(op[:],
                                            otsb[:, 128 * j:128 * (j + 1)],
                                            ident[:])
                        yt = yp.tile([128, 128], f32, tag="y")
                        nc.vector.tensor_scalar_mul(yt[:], op[:],
                                                    rcp[:, j:j + 1])
                        nc.sync.dma_start(yv[:, 4 * J + j, :], yt[:])
            rep_stack.close()
    nc.compile()
    return nc


def _host_consts():
    ident = np.eye(128, dtype=np.float32)
    kk = np.arange(128)[:, None]
    qq = np.arange(128)[None, :]
    mask = np.where(qq >= kk, 0.0, NEG).astype(np.float32)
    return ident, mask


def kernel(x, Wq, Wk, Wv):
    from concourse import bass_utils

    if "nc" not in _cache:
        _cache["nc"] = _build()
    nc = _cache["nc"]

    x = np.ascontiguousarray(x, dtype=np.float32)
    ident, mask = _host_consts()
    in_maps = []
    for c in range(NCORES):
        in_maps.append({
            "x": x[BPC * c:BPC * (c + 1)],
            "Wq": np.ascontiguousarray(Wq, dtype=np.float32),
            "Wk": np.ascontiguousarray(Wk, dtype=np.float32),
            "Wv": np.ascontiguousarray(Wv, dtype=np.float32),
            "ident": ident,
            "mask": mask,
        })
    res = bass_utils.run_bass_kernel_spmd(nc, in_maps,
                                          core_ids=list(range(NCORES)))
    _cache["last_results"] = res
    y = np.concatenate([res.results[c]["y"] for c in range(NCORES)], axis=0)
    return y
